# revision 20
# baseline (speedup 1.0000x reference)
"""DirichletMoE Trainium2 kernel (8 NeuronCores).

Strategy: the reference runs all 16 experts densely on all 8192 tokens, but
the output only depends on each token's top-2 gated experts.  We therefore:

  Phase 1 (device, data parallel): gating logits (x@gate_W, x@noise_W) in
     exact fp32 on 8 cores (1024 tokens each).
  Host: softplus/noisy-logits/top-2/softmax gates (exact fp32, mimicking the
     jax reference), aux_loss, and a counting-sort of (token, gate) pairs
     into per-expert capacity buckets.
  Phase 2 (device, expert parallel): 2 experts per core; each expert runs its
     gathered tokens through the 3-layer MLP with fp32r matmuls (full PE
     rate), fused bias rows, on-device softmax and gate weighting.
  Host: add each token's two expert contributions, renormalize p_hat,
     softplus/clip for alpha0.
"""

import numpy as np
from contextlib import ExitStack

import concourse.bacc as bacc
import concourse.mybir as mybir
import concourse.tile as tile
from concourse.bass_utils import run_bass_kernel_spmd
from concourse.masks import make_identity

F32 = mybir.dt.float32
F32R = mybir.dt.float32r
AF = mybir.ActivationFunctionType
ALU = mybir.AluOpType
AX = mybir.AxisListType

B, IN, HID, OUT, E = 8192, 512, 1024, 256, 16
TOPK = 2
NCORES = 8
EPC = E // NCORES           # experts per core
BPC = B // NCORES           # tokens per core (phase 1)
ALPHA0_INIT, ALPHA0_MIN, ALPHA0_MAX = 10.0, 1.0, 500.0

KIN = IN // 128             # 4  k-chunks of IN
KH = HID // 128             # 8  k-chunks of HID
OA = OUT + 2                # 258: softmax logits + alpha preact + pad
                            # (fp32r matmul needs an even moving free dim)

_cache = {}


# --------------------------------------------------------------------------
# Phase 1: gating logits.  Per core: x [BPC, IN] -> logits [32, BPC]
# (rows 0..15 = clean logits^T, rows 16..31 = noise logits^T)
# --------------------------------------------------------------------------
def _build_phase1(repeat=1):
    nc = bacc.Bacc("TRN2", target_bir_lowering=False, debug=False,
                   enable_asserts=False, num_devices=NCORES)
    x_d = nc.dram_tensor("x", [BPC, IN], F32, kind="ExternalInput")
    # [gate_W | noise_W] pre-chunked on host: [128, KIN, 32]
    gwn_d = nc.dram_tensor("gwn", [128, KIN * 32], F32, kind="ExternalInput")
    gnb_d = nc.dram_tensor("gnb", [32], F32, kind="ExternalInput")
    lg_d = nc.dram_tensor("logits", [32, BPC], F32, kind="ExternalOutput")

    with tile.TileContext(nc) as tc, ExitStack() as ctx:
        pool = ctx.enter_context(tc.tile_pool(name="sbuf", bufs=2))
        cpool = ctx.enter_context(tc.tile_pool(name="const", bufs=1))
        pt = ctx.enter_context(tc.tile_pool(name="pt", bufs=2, space="PSUM"))
        pl = ctx.enter_context(tc.tile_pool(name="pl", bufs=2, space="PSUM"))

        ident = cpool.tile([128, 128], F32)
        make_identity(nc, ident)
        gwn = cpool.tile([128, KIN, 32], F32)
        nc.sync.dma_start(gwn, gwn_d.ap().rearrange("p (k n) -> p k n", k=KIN))
        gnb = cpool.tile([32, 1], F32)
        nc.sync.dma_start(gnb, gnb_d.ap().rearrange("(b o) -> b o", o=1))
        lg = cpool.tile([32, BPC], F32)

        ntile = BPC // 128
        for _rep in range(repeat):
            for t in range(ntile):
                xt = pool.tile([128, IN], F32)
                nc.sync.dma_start(xt, x_d.ap()[t * 128:(t + 1) * 128, :])
                xT = pool.tile([128, KIN, 128], F32)
                for k in range(KIN):
                    ps = pt.tile([128, 128], F32)
                    nc.tensor.transpose(ps, xt[:, k * 128:(k + 1) * 128], ident)
                    nc.vector.tensor_copy(xT[:, k, :], ps)
                pls = pl.tile([32, 128], F32)
                for k in range(KIN):
                    nc.tensor.matmul(pls, gwn[:, k, :], xT[:, k, :],
                                     start=(k == 0), stop=(k == KIN - 1))
                nc.scalar.activation(lg[:, t * 128:(t + 1) * 128], pls,
                                     AF.Identity, bias=gnb, scale=1.0)
        nc.sync.dma_start(lg_d.ap(), lg)

    nc.compile()
    return nc


# --------------------------------------------------------------------------
# Phase 2: expert MLPs on gathered tokens.  Per core, EPC experts with
# capacity CAP each:
#   xg    [EPC, CAP, IN]    gathered tokens (zero padded)
#   w1    [EPC, 128, KIN*HID]   W1 pre-chunked (k-major)
#   w2    [EPC, 128, KH*HID]
#   wpa   [EPC, 128, KH*OA]     [Wp | Wa] pre-chunked
#   bpa   [EPC, OA]             [bp | ba]
#   b1c   [EPC, 128, KH]        b1 chunked (b1c[e,p,h] = b1[e, h*128+p])
#   b2c   [EPC, 128, KH]
#   gt    [EPC, 128, CAP//128]  gates (gt[e,p,s] = gate of slot s*128+p)
# outputs:
#   pw    [EPC, CAP, OUT]       gate * softmax(h2@Wp + bp)
#   a0p   [EPC, CAP//128, 128]  h2@Wa + ba   (slot s*128+p at [e, s, p])
# --------------------------------------------------------------------------
def _token_tiles(cap):
    # macro-tiles of 256..512 tokens (multiples of 128); fp32r matmuls run
    # at full rate only when the moving free dim is >= 256
    tts = []
    off = 0
    while off < cap:
        rem = cap - off
        t = min(512, rem)
        if rem - t == 128:
            t = min(512, rem) - 128
        tts.append((off, t))
        off += t
    assert all(t >= 256 or cap < 256 for _, t in tts)
    return tts


def _build_phase2(cap, repeat=1):
    ns = cap // 128        # token subtiles per expert
    tts = _token_tiles(cap)

    nc = bacc.Bacc("TRN2", target_bir_lowering=False, debug=False,
                   enable_asserts=False, num_devices=NCORES)
    # xgt: gathered tokens pre-transposed on host to [IN, cap] per expert.
    # f32r DRAM + plain HWDGE DMA: the PE rounds f32r operands on read, so
    # no casting (gpsimd) DMA or DVE rounding pass is needed.
    xgt_d = nc.dram_tensor("xgt", [EPC, IN, cap], F32R, kind="ExternalInput")
    w1_d = nc.dram_tensor("w1", [EPC, 128, KIN * HID], F32R, kind="ExternalInput")
    w2_d = nc.dram_tensor("w2", [EPC, 128, KH * HID], F32R, kind="ExternalInput")
    wpa_d = nc.dram_tensor("wpa", [EPC, 128, KH * OA], F32R, kind="ExternalInput")
    bpa_d = nc.dram_tensor("bpa", [EPC, OA], F32, kind="ExternalInput")
    b1c_d = nc.dram_tensor("b1c", [EPC, 128, KH], F32, kind="ExternalInput")
    b2c_d = nc.dram_tensor("b2c", [EPC, 128, KH], F32, kind="ExternalInput")
    gt_d = nc.dram_tensor("gt", [EPC, 128, ns], F32, kind="ExternalInput")
    pw_d = nc.dram_tensor("pw", [EPC, cap, OUT], F32, kind="ExternalOutput")
    a0_d = nc.dram_tensor("a0p", [EPC, ns, 128], F32, kind="ExternalOutput")

    with tile.TileContext(nc) as tc, ExitStack() as ctx:
        cpool = ctx.enter_context(tc.tile_pool(name="const", bufs=1))
        wpool = ctx.enter_context(tc.tile_pool(name="w", bufs=2))
        apool = ctx.enter_context(tc.tile_pool(name="act", bufs=2))
        hpool = ctx.enter_context(tc.tile_pool(name="h", bufs=1))
        spool = ctx.enter_context(tc.tile_pool(name="small", bufs=4))
        pmm = ctx.enter_context(tc.tile_pool(name="pmm", bufs=4, space="PSUM"))
        pd = ctx.enter_context(tc.tile_pool(name="pd", bufs=2, space="PSUM"))

        ones = cpool.tile([1, 128], F32)
        nc.vector.memset(ones, 1.0)

        for e in [e for _ in range(repeat) for e in range(EPC)]:
            # weights on the gpsimd SWDGE ring so the token (xgt) loads on
            # the SP HWDGE ring are not queued behind 7MB of weights
            w1 = wpool.tile([128, KIN, HID], F32R, tag="w1")
            nc.gpsimd.dma_start(
                w1, w1_d.ap()[e].rearrange("p (k n) -> p k n", k=KIN))
            w2 = wpool.tile([128, KH, HID], F32R, tag="w2")
            nc.gpsimd.dma_start(
                w2, w2_d.ap()[e].rearrange("p (k n) -> p k n", k=KH))
            wpa = wpool.tile([128, KH, OA], F32R, tag="wpa")
            nc.gpsimd.dma_start(
                wpa, wpa_d.ap()[e].rearrange("p (k n) -> p k n", k=KH))
            bpa = wpool.tile([1, OA], F32, tag="bpa")
            nc.sync.dma_start(bpa, bpa_d.ap()[e].rearrange("(b n) -> b n", b=1))
            b1c = wpool.tile([128, KH], F32, tag="b1c")
            nc.sync.dma_start(b1c, b1c_d.ap()[e])
            b2c = wpool.tile([128, KH], F32, tag="b2c")
            nc.sync.dma_start(b2c, b2c_d.ap()[e])
            gt = wpool.tile([128, ns], F32, tag="gt")
            nc.sync.dma_start(gt, gt_d.ap()[e])
            a0t = wpool.tile([128, ns], F32, tag="a0t")

            for (t0, T) in tts:
                S = T // 128
                # xT [128(in-in-chunk), k, T(tok)] straight from DRAM
                xT = apool.tile([128, KIN, 512], F32R, tag="xT")
                nc.sync.dma_start(
                    xT[:, :, :T],
                    xgt_d.ap()[e].rearrange("(k p) t -> p k t",
                                            p=128)[:, :, t0:t0 + T])
                # h1T = relu(W1^T x + b1)   [128, KH, T]
                h1T = hpool.tile([128, KH, 512], F32R, tag="h1T")
                for h in range(KH):
                    pb = pmm.tile([128, 512], F32, tag="pmm")
                    for k in range(KIN):
                        nc.tensor.matmul(pb[:, :T],
                                         w1[:, k, h * 128:(h + 1) * 128],
                                         xT[:, k, :T],
                                         start=(k == 0), stop=(k == KIN - 1))
                    # relu(x + b) on DVE (ACT cannot emit f32r-rounded output)
                    nc.vector.tensor_scalar(h1T[:, h, :T], pb[:, :T],
                                            b1c[:, h:h + 1], 0.0,
                                            op0=ALU.add, op1=ALU.max)
                # h2T = relu(W2^T h1 + b2)  [128, KH, T]
                h2T = hpool.tile([128, KH, 512], F32R, tag="h2T")
                for g in range(KH):
                    pc = pmm.tile([128, 512], F32, tag="pmm")
                    for h in range(KH):
                        nc.tensor.matmul(pc[:, :T],
                                         w2[:, h, g * 128:(g + 1) * 128],
                                         h1T[:, h, :T],
                                         start=(h == 0), stop=(h == KH - 1))
                    nc.vector.tensor_scalar(h2T[:, g, :T], pc[:, :T],
                                            b2c[:, g:g + 1], 0.0,
                                            op0=ALU.add, op1=ALU.max)
                # D: logits+alpha [tok, OA] per 128-token subtile
                for s in range(S):
                    gs = t0 // 128 + s
                    pdt = pd.tile([128, OA], F32)
                    for h in range(KH):
                        nc.tensor.matmul(pdt,
                                         h2T[:, h, s * 128:(s + 1) * 128],
                                         wpa[:, h, :],
                                         start=(h == 0), stop=False)
                    nc.tensor.matmul(pdt, ones, bpa, start=False, stop=True)
                    # softmax over [:, :OUT], weighted by gate * 1/sum
                    mx = spool.tile([128, 1], F32, tag="mx")
                    nc.vector.tensor_reduce(mx, pdt[:, :OUT], axis=AX.X,
                                            op=ALU.max)
                    nmx = spool.tile([128, 1], F32, tag="nmx")
                    nc.vector.tensor_scalar_mul(nmx, mx, -1.0)
                    pexp = apool.tile([128, OUT], F32, tag="pexp")
                    ssum = spool.tile([128, 1], F32, tag="ssum")
                    nc.scalar.activation(pexp, pdt[:, :OUT], AF.Exp,
                                         bias=nmx, scale=1.0, accum_out=ssum)
                    rec = spool.tile([128, 1], F32, tag="rec")
                    nc.vector.reciprocal(rec, ssum)
                    sc = spool.tile([128, 1], F32, tag="sc")
                    nc.vector.tensor_mul(sc, rec, gt[:, gs:gs + 1])
                    pwt = apool.tile([128, OUT], F32, tag="pwt")
                    nc.scalar.activation(pwt, pexp, AF.Copy, scale=sc)
                    nc.sync.dma_start(
                        pw_d.ap()[e, t0 + s * 128:t0 + (s + 1) * 128, :], pwt)
                    nc.vector.tensor_copy(a0t[:, gs:gs + 1],
                                          pdt[:, OUT:OUT + 1])
            nc.sync.dma_start(
                a0_d.ap()[e].rearrange("s p -> p s"), a0t)

    nc.compile()
    return nc


def _softplus(x):
    return np.logaddexp(x, np.float32(0.0)).astype(np.float32)


def kernel(x, noise, gate_W, gate_b, noise_W, noise_b,
           W1, b1, W2, b2, Wp, bp, Wa, ba):
    x = np.ascontiguousarray(np.asarray(x, np.float32))
    noise = np.asarray(noise, np.float32)
    gate_W = np.asarray(gate_W, np.float32)
    gate_b = np.asarray(gate_b, np.float32)
    noise_W = np.asarray(noise_W, np.float32)
    noise_b = np.asarray(noise_b, np.float32)
    W1 = np.asarray(W1, np.float32)
    b1 = np.asarray(b1, np.float32)
    W2 = np.asarray(W2, np.float32)
    b2 = np.asarray(b2, np.float32)
    Wp = np.asarray(Wp, np.float32)
    bp = np.asarray(bp, np.float32)
    Wa = np.asarray(Wa, np.float32)
    ba = np.asarray(ba, np.float32)

    # ---------------- phase 1: gating logits on device ----------------
    if "p1" not in _cache:
        _cache["p1"] = _build_phase1()
    nc1 = _cache["p1"]
    gwn = np.concatenate([gate_W, noise_W], axis=1)          # [IN, 32]
    gwn_pre = np.ascontiguousarray(
        gwn.reshape(KIN, 128, 32).transpose(1, 0, 2).reshape(128, KIN * 32))
    gnb = np.concatenate([gate_b, noise_b])                  # [32]
    maps1 = [{"x": x[c * BPC:(c + 1) * BPC],
              "gwn": gwn_pre, "gnb": gnb} for c in range(NCORES)]
    res1 = run_bass_kernel_spmd(nc1, maps1, list(range(NCORES)))
    lg = np.concatenate([res1.results[c]["logits"] for c in range(NCORES)],
                        axis=1)                              # [32, B]
    clean = np.ascontiguousarray(lg[:E].T)                   # [B, E]
    noisel = np.ascontiguousarray(lg[E:].T)                  # [B, E]

    # ---------------- host: routing (exact fp32, mimics jax) ----------
    noise_std = _softplus(noisel)
    noisy = (clean + noise * noise_std).astype(np.float32)
    rows = np.arange(B)
    i1 = np.argmax(noisy, axis=1)
    v1 = noisy[rows, i1]
    tmp = noisy.copy()
    tmp[rows, i1] = -np.inf
    i2 = np.argmax(tmp, axis=1)
    v2 = tmp[rows, i2]
    # softmax over [v1, v2] (v1 >= v2)
    e1 = np.exp(v1 - v1).astype(np.float32)                  # ones
    e2 = np.exp(v2 - v1).astype(np.float32)
    den = e1 + e2
    g1 = (e1 / den).astype(np.float32)
    g2 = (e2 / den).astype(np.float32)

    gates_full = np.zeros((B, E), np.float32)
    gates_full[rows, i1] = g1
    gates_full[rows, i2] = g2
    importance = gates_full.sum(axis=0, dtype=np.float32)
    load = (gates_full > 0).astype(np.float32).sum(axis=0, dtype=np.float32)

    def _std1(a):
        return np.std(a.astype(np.float32), ddof=1, dtype=np.float32)

    aux_loss = np.float32(
        _std1(importance) / (importance.mean(dtype=np.float32) + np.float32(1e-8))
        + _std1(load) / (load.mean(dtype=np.float32) + np.float32(1e-8)))

    # counting sort into per-expert buckets
    tok_all = np.concatenate([rows, rows])                   # [2B]
    exp_all = np.concatenate([i1, i2])
    g_all = np.concatenate([g1, g2])
    order = np.argsort(exp_all, kind="stable")
    tok_s, exp_s, g_s = tok_all[order], exp_all[order], g_all[order]
    counts = np.bincount(exp_all, minlength=E)
    maxload = int(counts.max())
    cap = max(1152, -(-maxload // 128) * 128)
    ends = np.cumsum(counts)
    starts = ends - counts
    slot = np.arange(2 * B) - starts[exp_s]                  # slot within bucket
    flat = exp_s * cap + slot                                # position in [E*cap]

    xg = np.zeros((E * cap, IN), np.float32)
    xg[flat] = x[tok_s]
    gflat = np.zeros(E * cap, np.float32)
    gflat[flat] = g_s
    # device wants tokens transposed: [E, IN, cap]
    xgt = np.ascontiguousarray(
        xg.reshape(E, cap, IN).transpose(0, 2, 1))

    # ---------------- phase 2: expert MLPs on device ------------------
    key = ("p2", cap)
    if key not in _cache:
        _cache[key] = _build_phase2(cap)
    nc2 = _cache[key]

    ns = cap // 128
    wpa_full = np.concatenate(
        [Wp, Wa[:, :, None], np.zeros((E, HID, 1), np.float32)], axis=2)
    bpa_full = np.concatenate(
        [bp, ba[:, None], np.zeros((E, 1), np.float32)], axis=1)  # [E, OA]
    w1_pre = np.ascontiguousarray(
        W1.reshape(E, KIN, 128, HID).transpose(0, 2, 1, 3)
        .reshape(E, 128, KIN * HID))
    w2_pre = np.ascontiguousarray(
        W2.reshape(E, KH, 128, HID).transpose(0, 2, 1, 3)
        .reshape(E, 128, KH * HID))
    wpa_pre = np.ascontiguousarray(
        wpa_full.reshape(E, KH, 128, OA).transpose(0, 2, 1, 3)
        .reshape(E, 128, KH * OA))
    b1c = np.ascontiguousarray(b1.reshape(E, KH, 128).transpose(0, 2, 1))
    b2c = np.ascontiguousarray(b2.reshape(E, KH, 128).transpose(0, 2, 1))
    gt_pre = np.ascontiguousarray(
        gflat.reshape(E, ns, 128).transpose(0, 2, 1))

    maps2 = []
    for c in range(NCORES):
        es = slice(c * EPC, (c + 1) * EPC)
        maps2.append({
            "xgt": xgt[es],
            "w1": w1_pre[es], "w2": w2_pre[es], "wpa": wpa_pre[es],
            "bpa": np.ascontiguousarray(bpa_full[es]),
            "b1c": b1c[es], "b2c": b2c[es], "gt": gt_pre[es],
        })
    res2 = run_bass_kernel_spmd(nc2, maps2, list(range(NCORES)))

    pw = np.concatenate(
        [res2.results[c]["pw"] for c in range(NCORES)], axis=0)   # [E,cap,OUT]
    a0p = np.concatenate(
        [res2.results[c]["a0p"] for c in range(NCORES)], axis=0)  # [E,ns,128]
    pw = pw.reshape(E * cap, OUT)
    a0p = a0p.reshape(E * cap)

    # ---------------- host: combine -----------------------------------
    a0 = np.clip(_softplus(a0p) + np.float32(ALPHA0_INIT),
                 np.float32(ALPHA0_MIN), np.float32(ALPHA0_MAX)).astype(np.float32)

    # position of each token's two (expert, slot) contributions
    pos = np.empty(2 * B, np.int64)
    pos[order] = flat
    pos1, pos2 = pos[:B], pos[B:]
    p_hat = pw[pos1] + pw[pos2]
    alpha0 = (g1 * a0[pos1] + g2 * a0[pos2]).astype(np.float32)
    p_hat = (p_hat / (p_hat.sum(axis=-1, keepdims=True) + np.float32(1e-8))
             ).astype(np.float32)

    return p_hat, alpha0, aux_loss
